# revision 20
# baseline (speedup 1.0000x reference)
"""Trainium2 Bass kernel: e3nn-style CudaTensorProduct (CG tensor product).

out[b, o] = sum_nnz cb * in1[b, i1] * in2[b, i2]

in1: [8192, 288] = 32 channels each of l1=0,1,2 (dims 1/3/5)
in2: [8192, 9]   = spherical harmonics l2=0..2
out: [8192, 2592]

Device formulation (per core, batch slice of 1024, data parallel over 8 cores):
  For each group g (l1=0,1,2) the CG coefficients are channel-independent.
  Let p = (g, m1, j) index the 81 = 9*(1+3+5) "product rows" and
  q = (g, o3) index the 81 = 9+27+45 output rows per channel.
    Z[p, (c, b)]   = in1[b, col(g,c,m1)] * in2[b, j]
    out[q, (c, b)] = sum_p T_all[p, q] * Z[p, (c, b)]

  Per 128-row batch tile (32 channels = 4 pairs of 1024 columns):
  - pairs 0,1 ("psum path"): pa = S.T @ in1a (PE replicate, fp16 rhs),
    z = pa * in2rep on DVE/GpSimd (PSUM read, 1x), z fp16.
  - pairs 2,3 ("sbuf path"): the 9->81 row replication is done on the HOST
    (in1b, fp16, read straight from HBM), so z = in1b * in2rep runs on DVE
    in all-SBUF fp16 2x mode and needs no PE matmul / PSUM bank.
  - all pairs: pb = T_all.T @ z (PE, lhsT fp32r), then PSUM->SBUF fp16
    convert (ACT for three pairs, DVE for one), one fp16 DMA store per tile.
  This balances every engine under the PE+conv envelope (~3.6us/tile).
  Host un-permutes the [q, (t, c, b)] device layout into [b, o] (pure data
  movement).  T_all / S and the output column permutation are derived from
  the COO tables (cb_vals, i1_idx, i2_idx, out_idx) passed in as inputs.
"""

from contextlib import ExitStack

import numpy as np

import concourse.bass as bass
import concourse.mybir as mybir
import concourse.tile as tile
from concourse import bacc
from concourse.bass_utils import run_bass_kernel_spmd

# ---- hardcoded problem geometry ----
B = 8192
DIM1 = 288
DIM2 = 9
CBH = 2592
NCORES = 8
BLOC = B // NCORES          # 1024 batch rows per core
PT = 128                    # partition tile (batch rows per tile)
NT = BLOC // PT             # 8 tiles per core
NCHAN = 32
NROW = 81                   # (g, m1, j) product rows
NQ = 81                     # (g, o3) output rows per channel
CHUNK_C = 4                 # channels per 512-col chunk
FREE = CHUNK_C * PT         # 512 = matmul moving-dim per chunk
NPAIR = 4                   # pairs (2 chunks = 8 channels) per tile
PAIRW = 2 * FREE            # 1024 cols per pair
NPS = 1                     # pair 0 via PE-replicate; 1,2,3 via host-replicate

# per group: (col offset in in1, 2*l1+1, gm1 row offset, q offset, D3)
GRP = [(0, 1, 0, 0, 9), (32, 3, 1, 9, 27), (128, 5, 4, 36, 45)]

F32 = mybir.dt.float32
F32R = mybir.dt.float32r
F16 = mybir.dt.float16

_cache: dict = {}


# --------------------------------------------------------------------------
# Tables from the COO inputs
# --------------------------------------------------------------------------
def _build_tables(cb_vals, i1_idx, i2_idx, out_idx):
    """Build T_all [81, 81], S [9, 81], colmap [81, 32] from the COO triple.

    T_all[p, q]: coefficient taking product row p=(g,m1,j) to output row
    q=(g,o3).  colmap[q, c]: the out column for output row q of channel c.
    Relies on (and verifies) the CG coefficients being channel-independent
    and the out-column order being consistent across channels.
    """
    cb = np.asarray(cb_vals, np.float64)
    i1 = np.asarray(i1_idx, np.int64)
    i2 = np.asarray(i2_idx, np.int64)
    oo = np.asarray(out_idx, np.int64)

    g = np.where(i1 < 32, 0, np.where(i1 < 128, 1, 2))
    rel = i1 - np.array([0, 32, 128])[g]
    width = np.array([1, 3, 5])[g]
    c = rel // width
    m1 = rel % width
    gm1 = np.array([0, 1, 4])[g] + m1
    p = gm1 * 9 + i2

    # distinct out columns per (g, c), sorted ascending -> rank k
    ocols: dict = {}
    for gg, cc, o in zip(g, c, oo):
        ocols.setdefault((int(gg), int(cc)), set()).add(int(o))
    rank: dict = {}
    for (gg, cc), s in ocols.items():
        d3 = GRP[gg][4]
        assert len(s) == d3, f"group {gg} chan {cc}: {len(s)} cols != {d3}"
        for k, o in enumerate(sorted(s)):
            rank[(gg, cc, o)] = k

    T_all = np.zeros((NROW, NQ), np.float64)
    have = np.zeros((NROW, NQ), bool)
    colmap = -np.ones((NQ, NCHAN), np.int64)
    for n in range(len(cb)):
        gg, cc = int(g[n]), int(c[n])
        q = GRP[gg][3] + rank[(gg, cc, int(oo[n]))]
        colmap[q, cc] = oo[n]
        if have[p[n], q]:
            assert abs(T_all[p[n], q] - cb[n]) < 1e-5, "CG not channel-uniform"
        else:
            T_all[p[n], q] = cb[n]
            have[p[n], q] = True
    assert (colmap >= 0).all()
    perm = colmap.reshape(-1)
    assert np.array_equal(np.sort(perm), np.arange(CBH)), "colmap not a perm"

    S = np.zeros((9, NROW), np.float32)
    for pp in range(NROW):
        S[pp // 9, pp] = 1.0
    return T_all.astype(np.float16), S.astype(np.float16), perm


# --------------------------------------------------------------------------
# Device kernel
# --------------------------------------------------------------------------
def _trace_module():
    nc = bacc.Bacc(trn_type="TRN2")
    # channels 0..15 (pairs 0,1 of each tile), 9 gm1 rows
    in1a = nc.dram_tensor("in1a", [9, NT * NPS * PAIRW], F16, kind="ExternalInput")
    # channels 16..31 (pairs 2,3), host-replicated to all 81 (gm1, j) rows
    in1b = nc.dram_tensor(
        "in1b", [NROW, NT * (NPAIR - NPS) * PAIRW], F16, kind="ExternalInput"
    )
    in2r = nc.dram_tensor("in2r", [NROW, BLOC], F16, kind="ExternalInput")
    tmat = nc.dram_tensor("tmat", [NROW, NQ], F16, kind="ExternalInput")
    smat = nc.dram_tensor("smat", [9, NROW], F16, kind="ExternalInput")
    out16 = nc.dram_tensor(
        "out16", [NQ, NT * NPAIR * PAIRW], F16, kind="ExternalOutput"
    )

    with tile.TileContext(nc) as tc, ExitStack() as ctx:
        _cg_body(ctx, tc, out16, in1a, in1b, in2r, tmat, smat)
    nc.compile()
    return nc


def _cg_body(ctx, tc, out16, in1a, in1b, in2r, tmat, smat):
    nc = tc.nc
    const = ctx.enter_context(tc.tile_pool(name="const", bufs=1))
    inpb = ctx.enter_context(tc.tile_pool(name="inpb", bufs=5))
    psa = ctx.enter_context(tc.tile_pool(name="psa", bufs=1, space="PSUM"))
    psb = ctx.enter_context(tc.tile_pool(name="psb", bufs=3, space="PSUM"))
    zp = ctx.enter_context(tc.tile_pool(name="zp", bufs=8))
    op = ctx.enter_context(tc.tile_pool(name="op", bufs=4))

    in1bv = in1b.ap().rearrange("p (t i c b) -> p t i c b", t=NT, i=NPAIR - NPS, c=2 * CHUNK_C)

    # in2 is needed by every multiply: issue its load first.  Tile 0's
    # sbuf-path input is split per-pair so pair 1 can start as early as
    # possible; later tiles stream whole with 3-deep prefetch.
    sb_in2 = const.tile([NROW, BLOC], F16)
    nc.sync.dma_start(out=sb_in2, in_=in2r.ap())

    in1bt = [None] * NT

    def _load_b(t):
        it = inpb.tile([NROW, NPAIR - NPS, 2 * CHUNK_C, PT], F16)
        nc.sync.dma_start(out=it, in_=in1bv[:, t])
        in1bt[t] = it

    # tile 0's inputs land in stream order: pair 3 (k=2) first, then pair 2
    # (k=1), tables, pool pair (k=0)
    it0 = inpb.tile([NROW, NPAIR - NPS, 2 * CHUNK_C, PT], F16)
    in1bt[0] = it0
    nc.sync.dma_start(out=it0[:, 2], in_=in1bv[:, 0, 2])
    nc.sync.dma_start(out=it0[:, 1], in_=in1bv[:, 0, 1])
    sb_s = const.tile([9, NROW], F16)
    nc.sync.dma_start(out=sb_s, in_=smat.ap())
    sb_t = const.tile([NROW, NQ], F16)
    nc.sync.dma_start(out=sb_t, in_=tmat.ap())

    # all psum-path in1 (small) in a single bulk DMA
    sb_a = const.tile([9, NT, NPS, 2 * CHUNK_C, PT], F16)
    nc.sync.dma_start(
        out=sb_a,
        in_=in1a.ap().rearrange(
            "g (t i c b) -> g t i c b", t=NT, i=NPS, c=2 * CHUNK_C
        ),
    )
    in1at = [sb_a[:, t] for t in range(NT)]
    nc.sync.dma_start(out=it0[:, 0], in_=in1bv[:, 0, 0])
    _load_b(1)
    _load_b(2)
    _load_b(3)

    lhs_s = sb_s[:]
    lhs_t = sb_t[:]

    def _mm1(t):
        # PE-replicate for pair 0 of tile t: pa = S.T @ in1a.  Hoisted one
        # tile ahead of its consumers so the next tile's DVE multiply never
        # waits on the PE finishing the current tile's (late) pool pair.
        pa = psa.tile([NROW, 2, CHUNK_C, PT], F32)
        for jj in range(2):
            nc.tensor.matmul(
                pa[:, jj],
                lhsT=lhs_s,
                rhs=in1at[t][:, 0, jj * CHUNK_C:(jj + 1) * CHUNK_C, :],
                start=True,
                stop=True,
            )
        return pa

    pa_next = _mm1(0)

    def _mm2(z, jj2=2):
        pb = psb.tile([NROW, 2, CHUNK_C, PT], F32)
        for jj in range(jj2):
            nc.tensor.matmul(
                pb[:, jj], lhsT=lhs_t, rhs=z[:, jj], start=True, stop=True
            )
        return pb

    def _store(out_sb, t, i):
        nc.sync.dma_start(
            out=out16.ap()[
                :, (t * NPAIR + i) * PAIRW:(t * NPAIR + i + 1) * PAIRW
            ],
            in_=out_sb[:, i],
        )

    for t in range(NT):
        pa_cur = pa_next
        if t + 4 < NT:
            _load_b(t + 4)
        out_sb = op.tile([NQ, NPAIR, 2, CHUNK_C, PT], F16)
        in2bb = (
            sb_in2[:, t * PT:(t + 1) * PT]
            .unsqueeze(1)
            .unsqueeze(1)
            .broadcast_to((NROW, 2, CHUNK_C, PT))
        )
        ins = [
            in1bt[t][:, k].rearrange("p (j c) b -> p j c b", j=2)
            for k in range(NPAIR - NPS)
        ]
        mult = mybir.AluOpType.mult

        last = t == NT - 1
        if last:
            # final tile: pool pair first so its late conv isn't the tail
            z1 = zp.tile([NROW, 2, CHUNK_C, PT], F16)
            nc.gpsimd.tensor_tensor(out=z1[:], in0=ins[0], in1=in2bb, op=mult)

        # multiplies, ordered so DVE's wait for mm2(p3) is filled by work:
        # DVE stream [p3, p2c0, p0, p3-conv]; Pool stream [p2c1, p1]
        z3 = zp.tile([NROW, 2, CHUNK_C, PT], F16)
        nc.vector.tensor_tensor(out=z3[:], in0=ins[2], in1=in2bb, op=mult)
        z2 = zp.tile([NROW, 2, CHUNK_C, PT], F16)
        nc.vector.tensor_tensor(
            out=z2[:, 0], in0=ins[1][:, 0], in1=in2bb[:, 0], op=mult
        )
        nc.gpsimd.tensor_tensor(
            out=z2[:, 1], in0=ins[1][:, 1], in1=in2bb[:, 1], op=mult
        )
        if not last:
            z1 = zp.tile([NROW, 2, CHUNK_C, PT], F16)
            nc.gpsimd.tensor_tensor(out=z1[:], in0=ins[0], in1=in2bb, op=mult)
        z0 = zp.tile([NROW, 2, CHUNK_C, PT], F16)
        nc.vector.tensor_tensor(out=z0[:], in0=pa_cur[:], in1=in2bb, op=mult)

        # contractions + converts + stores
        pb3 = _mm2(z3)
        if last:
            pb1 = _mm2(z1)
            nc.scalar.copy(out=out_sb[:, 1], in_=pb1[:])
            _store(out_sb, t, 1)
        pb2 = _mm2(z2)
        pb0 = _mm2(z0)
        nc.vector.tensor_copy(out=out_sb[:, 3], in_=pb3[:])
        _store(out_sb, t, 3)
        nc.scalar.copy(out=out_sb[:, 2], in_=pb2[:])
        _store(out_sb, t, 2)
        nc.scalar.copy(out=out_sb[:, 0], in_=pb0[:])
        _store(out_sb, t, 0)
        if t + 1 < NT:
            pa_next = _mm1(t + 1)
        if not last:
            pb1 = _mm2(z1)
            nc.scalar.copy(out=out_sb[:, 1], in_=pb1[:])
            _store(out_sb, t, 1)


def _get_module():
    if "nc" not in _cache:
        _cache["nc"] = _trace_module()
    return _cache["nc"]


# --------------------------------------------------------------------------
# Host glue
# --------------------------------------------------------------------------
def _prep_in1(in1):
    """in1 [B, 288] -> per-core (in1a [9, NT*2048] f16, in1b [81, NT*2048] f16).

    in1a: channels 0..15 in (gm1) x (t, pair, chunk-chan, b) layout.
    in1b: channels 16..31 replicated to rows p=(gm1, j), j=0..8.
    """
    g0 = in1[:, 0:32].T[None]                                  # [1, 32, B]
    g1 = in1[:, 32:128].reshape(B, 32, 3).transpose(2, 1, 0)   # [3, 32, B]
    g2 = in1[:, 128:288].reshape(B, 32, 5).transpose(2, 1, 0)  # [5, 32, B]
    r = np.concatenate([g0, g1, g2], axis=0).astype(np.float16)  # [9, 32, B]
    rep = r[np.arange(NROW) // 9]                              # [81, 32, B]
    ca = NPS * 8                  # channels on the PE-replicate path
    cores_a, cores_b = [], []
    for k in range(NCORES):
        ra = r[:, :ca, k * BLOC:(k + 1) * BLOC].reshape(9, ca, NT, PT)
        ra = ra.transpose(0, 2, 1, 3).reshape(9, NT * ca * PT)
        cores_a.append(np.ascontiguousarray(ra))
        rb = rep[:, ca:, k * BLOC:(k + 1) * BLOC].reshape(NROW, 32 - ca, NT, PT)
        rb = rb.transpose(0, 2, 1, 3).reshape(NROW, NT * (32 - ca) * PT)
        cores_b.append(np.ascontiguousarray(rb))
    return cores_a, cores_b


def _prep_in2(in2):
    rep = in2.T[np.arange(NROW) % 9].astype(np.float16)        # [81, B]
    return [
        np.ascontiguousarray(rep[:, k * BLOC:(k + 1) * BLOC])
        for k in range(NCORES)
    ]


def kernel(in1, in2, cb_vals, i1_idx, i2_idx, out_idx, **run_kwargs):
    in1 = np.asarray(in1, np.float32)
    in2 = np.asarray(in2, np.float32)
    assert in1.shape == (B, DIM1) and in2.shape == (B, DIM2)

    if "tables" not in _cache:
        _cache["tables"] = _build_tables(cb_vals, i1_idx, i2_idx, out_idx)
    t_all, s_mat, perm = _cache["tables"]

    nc = _get_module()
    in1a_cores, in1b_cores = _prep_in1(in1)
    in2_cores = _prep_in2(in2)
    in_maps = [
        {
            "in1a": in1a_cores[k],
            "in1b": in1b_cores[k],
            "in2r": in2_cores[k],
            "tmat": t_all,
            "smat": s_mat,
        }
        for k in range(NCORES)
    ]
    res = run_bass_kernel_spmd(nc, in_maps, core_ids=list(range(NCORES)), **run_kwargs)
    _cache["last_results"] = res

    out = np.empty((B, CBH), np.float32)
    for k in range(NCORES):
        od = (
            np.asarray(res.results[k]["out16"])
            .astype(np.float32)
            .reshape(NQ, NT, NCHAN, PT)
        )
        oc = od.transpose(1, 3, 0, 2).reshape(BLOC, NQ * NCHAN)
        out[k * BLOC:(k + 1) * BLOC, perm] = oc
    return out
